# revision 39
# baseline (speedup 1.0000x reference)
"""Trainium2 Bass kernel for nn_DistanceLayer (shapelet min-distance).

reference semantics:
  x: (512, 1, 2048), shapelets: (128, 1, 64)
  patches = sliding windows of x (len 64, stride 1), mean-centered
  out[b, s] = min_p ||patch(b, p) - shapelets[s]||_2          -> (512, 128)

Math:
  With centered shapelets  s~ = sh - mean_l(sh):
    (w - mean(w)) . sh = w . s~
    d2[b,s,p] = A[b,p] + s2[s] - 2 w . s~
  where A = sum(w^2) - (sum w)^2/L and s2 = ||sh||^2.  min_p commutes with
  sqrt and s2 is constant over p, so PSUM only needs  A - 2 w.s~  and the
  drain is a pure min-reduce.

Layout (v2 - A embedded in the moving tile):
  Each sample's timeline is split into two overlapping halves of 1056
  samples (h=0: x[0:1056], h=1: x[992:2048]) so stats and windows use all
  128 partitions.  Windows p_local = 32*u + r with u in [0,32), r in
  [0,32).  Moving tile X2[k, u, bh]:
    rows  0..95 : x16h[bh, 32u + k]          (x data, zero-padded at edge)
    rows 96..127: A2h[bh, 32u + (k - 96)]   (window energies; BIG = masked)
  Weights W[k, r, s] = -2 s~[s, k - r] for r <= k < r+64, W[96+r, r, s]=1,
  else 0.  One K=128 matmul per (r, column-chunk) computes A - 2 w.s~
  directly -- no second A-fold matmul sweep (that halves PE time vs v1).
  Invalid windows (p_local > 992) get A = BIG so the min ignores them.

  Drain: running min per (r, half-tile) off PSUM, split two ways for
  engine balance: DVE fp32 TT-min straight off PSUM (1x), and ACT fp16
  cast-copy + DVE TT-min at 2x_1p.  (GPSIMD cannot run TensorTensor on
  this ISA build and has no PSUM port, so two channels is the maximum.)

Data-parallel over 8 NeuronCores: 64 samples each, shapelets replicated.
"""

import os
import sys

import numpy as np

for _p in ("/root/.axon_site/_ro/trn_rl_repo", "/opt/trn_rl_repo"):
    if os.path.isdir(_p) and _p not in sys.path:
        sys.path.append(_p)

B, C, T = 512, 1, 2048
S, L = 128, 64
NCORES = 8
BPC = B // NCORES          # samples per core = 64
P = T - L + 1              # 1985 windows
HW_, HOV = 992, 1056       # half stride / half width (windows 0..992 local)
NU, NR = 32, 32            # u-columns and r-shifts per half
NBH = 2 * BPC              # 128 partition rows (b, h)
NCOL = NU * NBH            # 4096 PSUM columns per pass
PL = HW_ + 1               # valid local windows per half = 993
BIG = 30000.0              # mask for invalid windows (fp16-safe)

# Drain path per unit (r, half-tile): 0 = DVE fp32 TT-min straight off
# PSUM (1x); 1 = ACT fp16 cast-copy + DVE TT-min at 2x_1p.  64 units.
_STATE = {}

_FLAGS = {"drain": True, "mains": True, "pool": False}
for _k in list(_FLAGS):
    _v = os.environ.get(f"K_{_k.upper()}")
    if _v is not None:
        _FLAGS[_k] = _v not in ("0", "false", "False")
_XCOUNT = int(os.environ.get("K_XCOUNT", "3"))  # DVE-direct units per 16


def _drain_paths():
    # 0 = DVE fp32 TT-min off PSUM, 1 = ACT fp16 cast + DVE fp16 TT-min.
    # _XCOUNT of every 16 units go to the direct-DVE path (engine balance).
    pat = [1] * 16
    for i in range(_XCOUNT):
        pat[(i * 16 // _XCOUNT + 2) % 16] = 0
    return [pat[i % len(pat)] for i in range(NR * 2)]


def _build(nc, reps=1):
    import concourse.tile as tile
    from concourse import mybir

    f32 = mybir.dt.float32
    f16 = mybir.dt.float16
    OP = mybir.AluOpType
    AF = mybir.ActivationFunctionType

    x16_d = nc.dram_tensor("x16", [BPC, T], f16, kind="ExternalInput").ap()
    wz_d = nc.dram_tensor("wz", [128, NR, S], f16, kind="ExternalInput").ap()
    s2_d = nc.dram_tensor("s2v", [S, 1], f32, kind="ExternalInput").ap()
    id_d = nc.dram_tensor("ident", [128, 128], f32, kind="ExternalInput").ap()
    out_d = nc.dram_tensor("out", [BPC, S], f32, kind="ExternalOutput").ap()

    paths = _drain_paths()

    with tile.TileContext(nc) as tc:
      for _it in range(reps):
        with tc.tile_pool(name=f"const{_it}", bufs=1) as constp, \
             tc.tile_pool(name=f"big{_it}", bufs=1) as bigp, \
             tc.tile_pool(name=f"drain{_it}", bufs=3) as drp:

            ident = constp.tile([128, 128], f32)
            nc.scalar.dma_start(ident[:], id_d[:])
            # fp16 identity: with fp16 data, transposes stream at 1 cyc/row
            # (vs 2 for fp32) and the 1.0s are exact.
            identb = constp.tile([128, 128], f16)
            nc.scalar.mul(identb[:], ident[:], 1.0)
            s2 = constp.tile([S, 1], f32)
            nc.scalar.dma_start(s2[:], s2_d[:])

            # x16h[b*2+h, t] = x[b, 992h + t], t in [0, 1056); fp16 feeds
            # both the PE (transposes/mains) and the stats (the scan's
            # internal state is fp32 regardless of operand dtype)
            x16h = bigp.tile([NBH, HOV], f16)
            x16hv = x16h[:].rearrange("(b h) t -> b h t", h=2)
            nc.sync.dma_start(x16hv[:, 0], x16_d[:, 0:HOV])
            nc.sync.dma_start(x16hv[:, 1], x16_d[:, HW_:HW_ + HOV])

            Wz = bigp.tile([128, NR, S], f16)
            nc.scalar.dma_start(Wz[:, 0:8], wz_d[:, 0:8])
            nc.gpsimd.dma_start(Wz[:, 8:16], wz_d[:, 8:16])
            nc.sync.dma_start(Wz[:, 16:24], wz_d[:, 16:24])
            nc.gpsimd.dma_start(Wz[:, 24:32], wz_d[:, 24:32])

            X2 = bigp.tile([128, NU, NBH], f16)

            # ---- sliding-window stats:  A = sum w^2 - (sum w)^2 / L
            sq = bigp.tile([NBH, HOV], f32)
            nc.scalar.activation(sq[:], x16h[:], AF.Square)
            cs = bigp.tile([NBH, HOV + 1], f32)
            cs2 = bigp.tile([NBH, HOV + 1], f32)
            nc.vector.memset(cs[:, 0:1], 0.0)
            nc.vector.memset(cs2[:, 0:1], 0.0)
            nc.vector.tensor_tensor_scan(cs[:, 1:HOV + 1], x16h[:], x16h[:],
                                         0.0, OP.add, OP.bypass)
            nc.vector.tensor_tensor_scan(cs2[:, 1:HOV + 1], sq[:], sq[:],
                                         0.0, OP.add, OP.bypass)
            sw = bigp.tile([NBH, PL], f32)
            nc.vector.tensor_sub(sw[:], cs[:, L:L + PL], cs[:, 0:PL])
            Ah = bigp.tile([NBH, PL], f32)
            nc.vector.tensor_sub(Ah[:], cs2[:, L:L + PL], cs2[:, 0:PL])
            # sw^2/L = Square(sw/8) since L = 64
            swsq = bigp.tile([NBH, PL], f32)
            nc.scalar.activation(swsq[:], sw[:], AF.Square, scale=1.0 / 8.0)
            # final A in fp16 (feeds the fp16 moving tile), BIG-padded
            Ah16 = bigp.tile([NBH, NU * 32], f16)
            nc.vector.memset(Ah16[:, PL:NU * 32], BIG)
            nc.vector.tensor_sub(Ah16[:, 0:PL], Ah[:], swsq[:])

            # fp16 running-min accumulator (both drain paths feed the DVE,
            # so one accumulator suffices; per-half chains are independent)
            maccA = bigp.tile([S, NU, NBH], f16)
            nc.gpsimd.memset(maccA[:], BIG * 2)

            # ---- build the moving tile X2 with PE transposes
            with tc.tile_pool(name=f"psTx{_it}", bufs=2, space="PSUM") as psTx, \
                 tc.tile_pool(name=f"psTa{_it}", bufs=2, space="PSUM") as psTa:
                # x rows: one transpose per u of x16h[:, 32u : 32u+96]
                for g in range(4):          # groups of 8 u
                    pt = psTx.tile([96, 8, NBH], f16, tag="tx")
                    nun = 8 if g < 3 else 7
                    for du in range(nun):
                        u = 8 * g + du
                        nc.tensor.transpose(pt[0:96, du],
                                            x16h[:, 32 * u:32 * u + 96],
                                            identb[:])
                    if g < 3:
                        if g % 2 == 0:
                            nc.scalar.mul(X2[0:96, 8 * g:8 * g + 8, :],
                                          pt[:], 1.0)
                        else:
                            nc.vector.tensor_copy(
                                X2[0:96, 8 * g:8 * g + 8, :], pt[:])
                    else:
                        nc.scalar.mul(X2[0:96, 24:31, :], pt[:, 0:7], 1.0)
                # last u (31): only rows 0..63 exist (x16h is 1056 wide)
                ptl = psTx.tile([96, 8, NBH], f16, tag="tx")
                nc.tensor.transpose(ptl[0:64, 0],
                                    x16h[:, 992:1056], identb[:])
                nc.vector.tensor_copy(X2[0:64, 31, :], ptl[0:64, 0])
                # zero-fill rows 64..95 of u=31 (DMA from Wz zeros,
                # rows 64..95 of any r are structurally zero)
                nc.sync.dma_start(X2[64:96, 31:32, :], Wz[64:96, 0:1, :])

                # A rows: one transpose per u of Ah[:, 32u : 32u+32]
                for g in range(4):
                    pa = psTa.tile([32, 8, NBH], f16, tag="ta")
                    for du in range(8):
                        u = 8 * g + du
                        nc.tensor.transpose(pa[:, du],
                                            Ah16[:, 32 * u:32 * u + 32],
                                            identb[:])
                    nc.scalar.mul(X2[96:128, 8 * g:8 * g + 8, :],
                                  pa[:], 1.0)

            # ---- main sweep over r: A - 2 w.s~ into PSUM, min-reduce out
            with tc.tile_pool(name=f"psB{_it}", bufs=2, space="PSUM") as psB:
                for r in range(NR):
                    for t in range(2):      # half-tiles of 16 u / 2048 cols
                        # u=31 only holds the edge window p_local=992 at r=0;
                        # for r>0 skip its columns entirely (PE + drain).
                        nu = 16 if (t == 0 or r == 0) else 15
                        ps = psB.tile([S, 16, NBH], f32, tag="ps")
                        if _FLAGS["mains"]:
                            for cc in range(4):
                                u0 = 16 * t + 4 * cc
                                du = min(4, nu - 4 * cc)
                                nc.tensor.matmul(
                                    ps[:, 4 * cc:4 * cc + du, :],
                                    Wz[:, r, :],
                                    X2[:, u0:u0 + du, :],
                                    start=True, stop=True)
                        if not _FLAGS["drain"]:
                            continue
                        path = paths[2 * r + t]
                        mt = maccA[:, 16 * t:16 * t + nu, :]
                        if path == 0:
                            nc.vector.tensor_tensor(mt, ps[:, 0:nu, :], mt,
                                                    OP.min)
                        else:
                            sb16 = drp.tile([S, 16, NBH], f16)
                            nc.scalar.mul(sb16[:, 0:nu, :], ps[:, 0:nu, :],
                                          1.0)
                            nc.vector.tensor_tensor(mt, sb16[:, 0:nu, :], mt,
                                                    OP.min)

                # ---- finish: tree-min over u, + s2, sqrt
                m16 = maccA[:].rearrange("p u b -> p (u b)")
                for half in (2048, 1024, 512, 256, 128):
                    nc.vector.tensor_tensor(m16[:, 0:half],
                                            m16[:, half:2 * half],
                                            m16[:, 0:half], OP.min)
                # m16[:, 0:128] = min over u, laid out [s, b*2+h]
                mn = constp.tile([S, BPC], f32)
                nc.vector.tensor_tensor(
                    mn[:], maccA[:, 0, 0:NBH:2], maccA[:, 0, 1:NBH:2], OP.min)
                nc.vector.tensor_scalar(mn[:], mn[:], s2[:], 0.0,
                                        OP.add, OP.max)
                res = constp.tile([S, BPC], f32)
                nc.scalar.activation(res[:], mn[:], AF.Sqrt)

            with tc.tile_pool(name=f"psC{_it}", bufs=1, space="PSUM") as psC:
                po = psC.tile([BPC, S], f32)
                nc.tensor.transpose(po[:], res[:], ident[:])
                outsb = constp.tile([BPC, S], f32)
                nc.scalar.mul(outsb[:], po[:], 1.0)
                nc.sync.dma_start(out_d[:], outsb[:])


def _wz_np(sh):
    # sh: (S, L) float32 -> Wz (128, NR, S) fp16:
    #   Wz[k, r, s] = -2 s~[s, k-r] for r <= k < r+64;  Wz[96+r, r, s] = 1
    st = -2.0 * (sh - sh.mean(axis=1, keepdims=True))      # (S, L)
    wz = np.zeros((128, NR, S), dtype=np.float32)
    for r in range(NR):
        wz[r:r + 64, r, :] = st.T
        wz[96 + r, r, :] = 1.0
    return wz.astype(np.float16)


def _get_nc():
    if "nc" not in _STATE:
        from concourse import bacc
        nc = bacc.Bacc("TRN2", target_bir_lowering=False, debug=False,
                       num_devices=NCORES)
        _build(nc)
        nc.compile()
        _STATE["nc"] = nc
    return _STATE["nc"]


def _in_maps(x, shapelets):
    x = np.ascontiguousarray(np.asarray(x, dtype=np.float32)).reshape(B, T)
    sh = np.ascontiguousarray(
        np.asarray(shapelets, dtype=np.float32)).reshape(S, L)
    wz = _wz_np(sh)
    s2v = (sh * sh).sum(axis=1, dtype=np.float32).reshape(S, 1)
    ident = np.eye(128, dtype=np.float32)
    x16 = x.astype(np.float16)
    return [{"x16": x16[i * BPC:(i + 1) * BPC], "wz": wz, "s2v": s2v,
             "ident": ident} for i in range(NCORES)]


def kernel(x, shapelets):
    from concourse.bass_utils import run_bass_kernel_spmd
    nc = _get_nc()
    res = run_bass_kernel_spmd(nc, _in_maps(x, shapelets),
                               list(range(NCORES))).results
    return np.concatenate([res[i]["out"] for i in range(NCORES)], axis=0)


if __name__ == "__main__":
    rng = np.random.default_rng(0)
    x = rng.standard_normal((B, C, T)).astype(np.float32)
    sh = rng.standard_normal((S, C, L)).astype(np.float32)
    out = kernel(x, sh)
    print("out", out.shape, out.dtype, float(out.min()), float(out.max()))
